# revision 37
# baseline (speedup 1.0000x reference)
"""Multi-head attention (B=4, S=2048, D=1024, H=16, DH=64) on 8 trn2 cores.

Strategy (v1: minimize host<->device traffic over the axon tunnel):
  - Host uploads x sharded by tokens: core c gets rows [c*1024, (c+1)*1024)
    of x.reshape(8192, 1024), cast to bf16 (16 MB total instead of 256 MB
    replicated fp32).
  - On-device AllGather reassembles the full bf16 x on every core.
  - Compute is tensor-parallel over heads (core c owns heads 2c, 2c+1),
    identical to the proven baseline kernel: qkv projection feature-major,
    scoresT = kT^T.qT per head/q-tile, exp on ACT engine, [ones|v]^T@attnT
    gives AV and the softmax denominator in one matmul, out-projection
    partial per core.
  - Partials (bf16) are ReduceScattered on device over the token axis; each
    core adds b_out and writes a disjoint [1024, 1024] bf16 output shard.
  - Host downloads 16 MB bf16, upcasts, reshapes. No host-side reduction.
  - The jit'd executable is built once and cached; weights are uploaded to
    device once and reused across calls (re-uploaded if contents change).

All matmuls run in float32r (tf32-like); transposes feed bf16 through the
PE with a bf16 identity (exact passthrough). PSUM accumulate is fp32.
"""

import numpy as np
import ml_dtypes

import jax
from jax.sharding import Mesh, PartitionSpec, NamedSharding
from jax.experimental.shard_map import shard_map

import concourse.bacc as bacc
import concourse.mybir as mybir
import concourse.tile as tile
from concourse.bass2jax import (
    _bass_exec_p,
    install_neuronx_cc_hook,
    partition_id_tensor,
)
from concourse.masks import make_identity

B, S, D, H, DH = 4, 2048, 1024, 16, 64
HPC = 2                      # heads per core
NCORES = 8
TOK = B * S                  # 8192 flattened tokens
TPC = TOK // NCORES          # 1024 tokens per core (I/O sharding)
F = 3 * HPC * DH             # 384 qkv features per core
SCALE = DH ** -0.5
P = 128
TT = 256                     # token tile for qkv projection
NTT = S // TT                # 8 token tiles per batch
QT = 256                     # q tile for attention
NQT = S // QT                # 8
NKB = S // P                 # 16 k blocks
NDC = D // P                 # 8 contraction chunks
NTB = S // P                 # 16 token blocks for proj

F32 = mybir.dt.float32
F32R = mybir.dt.float32r
BF16 = mybir.dt.bfloat16
F16 = mybir.dt.float16
I8 = mybir.dt.int8
GROUPS = [list(range(NCORES))]
MAGIC = 12582912.0           # 1.5 * 2**23: fp32 add/sub rounds to nearest int

X_INT8 = True                # upload x as int8 + per-token group scales
NG = 16                      # scale groups per token (64 features each)
GS = D // NG                 # 64
XW = D + 4 * NG if X_INT8 else D   # packed row width (int8 payload + scales)


def _build(collectives=True, repeat=1):
    nc = bacc.Bacc("TRN2", debug=False, num_devices=NCORES)

    if X_INT8:
        x_d = nc.dram_tensor("x_shard", [TPC, XW], I8, kind="ExternalInput")
    else:
        x_d = nc.dram_tensor("x_shard", [TPC, D], F16, kind="ExternalInput")
    wq_d = nc.dram_tensor("w_qkv_shard", [D, F], F32R, kind="ExternalInput")
    bq_d = nc.dram_tensor("b_qkv_shard", [F], F32, kind="ExternalInput")
    wo0_d = nc.dram_tensor("w_out0", [DH, D], F32R, kind="ExternalInput")
    wo1_d = nc.dram_tensor("w_out1", [DH, D], F32R, kind="ExternalInput")
    bo_d = nc.dram_tensor("b_out", [1, D], F32, kind="ExternalInput")
    # single packed output: [:, :D] int8 payload, [:, D:D+4] fp32 scale bitcast
    out_d = nc.dram_tensor("outp", [TPC, D + 4], I8, kind="ExternalOutput")

    with tile.TileContext(nc) as tc:
        with (
            tc.tile_pool(name="dram", bufs=1, space="DRAM") as dram,
            tc.tile_pool(name="const", bufs=1) as constp,
            tc.tile_pool(name="xp", bufs=2) as xp,
            tc.tile_pool(name="xtp", bufs=2) as xtp,
            tc.tile_pool(name="qkvp", bufs=1) as qkvp,
            tc.tile_pool(name="v1p", bufs=2) as v1p,
            tc.tile_pool(name="attp", bufs=2) as attp,
            tc.tile_pool(name="hp", bufs=2) as hp,
            tc.tile_pool(name="rp", bufs=3) as rp,
            tc.tile_pool(name="outsp", bufs=3) as outsp,
            tc.tile_pool(name="ps_t", bufs=1, space="PSUM") as ps_t,
            tc.tile_pool(name="ps_tb", bufs=1, space="PSUM") as ps_tb,
            tc.tile_pool(name="ps_mm", bufs=2, space="PSUM") as ps_mm,
            tc.tile_pool(name="ps_sc", bufs=2, space="PSUM") as ps_sc,
            tc.tile_pool(name="ps_av", bufs=2, space="PSUM") as ps_av,
        ):
            # ---- DRAM bounce buffers for collectives ----
            if X_INT8:
                xin_b = dram.tile([TPC, XW], I8)
                xg = dram.tile([TOK, XW], I8)     # gathered x (+packed scales)
            else:
                xin_b = dram.tile([TPC, D], F16)
                xg = dram.tile([TOK, D], F16)     # gathered x, all tokens
            part = dram.tile([TOK, D], BF16)      # this core's out-proj partial
            rs_out = dram.tile([TPC, D], BF16)    # reduce-scattered output shard

            nc.gpsimd.dma_start(xin_b[:], x_d.ap())
            if collectives:
                nc.gpsimd.collective_compute(
                    "AllGather",
                    mybir.AluOpType.bypass,
                    replica_groups=GROUPS,
                    ins=[xin_b.opt()],
                    outs=[xg.opt()],
                )
            else:
                nc.gpsimd.dma_start(xg[0:TPC, :], xin_b[:])

            # ---- constants ----
            wq_sb = constp.tile([P, NDC, F], F32R, tag="wq")
            nc.sync.dma_start(
                out=wq_sb[:], in_=wq_d.ap().rearrange("(c p) f -> p c f", p=P)
            )
            bq_sb = constp.tile([P, 3], F32, tag="bq")
            nc.sync.dma_start(
                out=bq_sb[:], in_=bq_d.ap().rearrange("(j p) -> p j", p=P)
            )
            wo_sb = [
                constp.tile([DH, D], F32R, tag=f"wo{h}", name=f"wo{h}")
                for h in range(HPC)
            ]
            nc.sync.dma_start(out=wo_sb[0][:], in_=wo0_d.ap())
            nc.sync.dma_start(out=wo_sb[1][:], in_=wo1_d.ap())
            ident = constp.tile([P, P], F16, tag="ident")
            make_identity(nc, ident[:])
            ident32 = constp.tile([P, P], F32, tag="ident32")
            make_identity(nc, ident32[:])
            ones_c = constp.tile([P, NKB], F32, tag="ones")
            nc.vector.memset(ones_c[:], 1.0)
            bo1 = constp.tile([1, D], F32, tag="bo1")
            nc.sync.dma_start(out=bo1[:], in_=bo_d.ap())
            bo_sb = constp.tile([P, D], F32, tag="bo")
            nc.gpsimd.partition_broadcast(bo_sb[:], bo1[0:1, :], channels=P)

            import itertools
            for rep, b in itertools.product(range(repeat), range(B)):
                # ---- qkv projection for batch b (feat-major output) ----
                qkvT = [
                    qkvp.tile(
                        [P, S], F32R if j < 2 else F32,
                        tag=f"qkvT{j}", name=f"qkvT{j}_{b}",
                    )
                    for j in range(3)
                ]  # q, k, v ; rows = 2 heads x 64
                for tt in range(NTT):
                    x_t = xp.tile([P, TT // P, D], F16, tag="x")
                    rows = slice(b * S + tt * TT, b * S + (tt + 1) * TT)
                    if X_INT8:
                        xq_t = xp.tile([P, TT // P, D], I8, tag="xq")
                        nc.sync.dma_start(
                            out=xq_t[:],
                            in_=xg[rows, 0:D].rearrange(
                                "(blk p) d -> p blk d", p=P
                            ),
                        )
                        sc_t = xp.tile([P, TT // P, NG], F32, tag="xsc")
                        nc.sync.dma_start(
                            out=sc_t[:],
                            in_=xg[rows, D:XW]
                            .rearrange("(blk p) d -> p blk d", p=P)
                            .bitcast(F32),
                        )
                        for blk in range(TT // P):
                            for g in range(NG):
                                nc.vector.tensor_scalar_mul(
                                    x_t[:, blk, g * GS : (g + 1) * GS],
                                    xq_t[:, blk, g * GS : (g + 1) * GS],
                                    sc_t[:, blk, g : g + 1],
                                )
                    else:
                        nc.sync.dma_start(
                            out=x_t[:],
                            in_=xg[rows, :].rearrange(
                                "(blk p) d -> p blk d", p=P
                            ),
                        )
                    xT = xtp.tile([P, NDC, TT], F32R, tag="xT")
                    for blk in range(TT // P):
                        for dc4 in range(NDC // 4):
                            tp = ps_tb.tile([P, 4, P], F16, tag="pstb")
                            for j in range(4):
                                dc = dc4 * 4 + j
                                nc.tensor.transpose(
                                    tp[:, j, :],
                                    x_t[:, blk, dc * P : (dc + 1) * P],
                                    ident[:],
                                )
                            nc.vector.tensor_copy(
                                xT[:, dc4 * 4 : (dc4 + 1) * 4, blk * P : (blk + 1) * P],
                                tp[:],
                            )
                    for ft in range(3):
                        mm = ps_mm.tile([P, TT], F32, tag="mm")
                        for dc in range(NDC):
                            nc.tensor.matmul(
                                mm[:],
                                wq_sb[:, dc, ft * P : (ft + 1) * P],
                                xT[:, dc, :],
                                start=(dc == 0),
                                stop=(dc == NDC - 1),
                            )
                        nc.vector.tensor_scalar_add(
                            qkvT[ft][:, tt * TT : (tt + 1) * TT],
                            mm[:],
                            bq_sb[:, ft : ft + 1],
                        )
                qT, kT, vT = qkvT

                # ---- v1 = [v | ones] token-major per head ----
                v1 = []
                for h in range(HPC):
                    v1_h = v1p.tile([P, NKB, DH + 1], F32R, tag="v1", name=f"v1_{b}_{h}")
                    nc.vector.tensor_copy(v1_h[:, :, DH], ones_c[:])
                    for kb8 in range(NKB // 8):
                        tp = ps_t.tile([P, 8, DH], F32, tag="pst")
                        for j in range(8):
                            kb = kb8 * 8 + j
                            nc.tensor.transpose(
                                tp[:, j, :],
                                vT[h * DH : (h + 1) * DH, kb * P : (kb + 1) * P],
                                ident32[h * DH : (h + 1) * DH, h * DH : (h + 1) * DH],
                            )
                        nc.vector.tensor_copy(
                            v1_h[:, kb8 * 8 : (kb8 + 1) * 8, 0:DH], tp[:]
                        )
                    v1.append(v1_h)

                # ---- attention per head / q-tile ----
                headsT = [
                    hp.tile([DH, S], F32R, tag=f"headsT{h}", name=f"headsT{h}_{b}")
                    for h in range(HPC)
                ]
                for h in range(HPC):
                    hs = slice(h * DH, (h + 1) * DH)
                    for qt in range(NQT):
                        qs = slice(qt * QT, (qt + 1) * QT)
                        attnT = attp.tile([P, NKB, QT], F32R, tag="attnT")
                        for kq in range(NKB // 2):
                            sc = ps_sc.tile([P, 2, QT], F32, tag="sc")
                            for j in range(2):
                                kc = kq * 2 + j
                                nc.tensor.matmul(
                                    sc[:, j, :],
                                    kT[hs, kc * P : (kc + 1) * P],
                                    qT[hs, qs],
                                    start=True,
                                    stop=True,
                                )
                            nc.scalar.activation(
                                attnT[:, kq * 2 : (kq + 1) * 2, :],
                                sc[:],
                                mybir.ActivationFunctionType.Exp,
                                bias=0.0,
                                scale=float(SCALE),
                            )
                        av = ps_av.tile([DH + 1, QT], F32, tag="av")
                        for kc in range(NKB):
                            nc.tensor.matmul(
                                av[:],
                                v1[h][:, kc, :],
                                attnT[:, kc, :],
                                start=(kc == 0),
                                stop=(kc == NKB - 1),
                            )
                        recip = rp.tile([DH + 1, QT], F32, tag="recip")
                        nc.vector.reciprocal(
                            recip[DH : DH + 1, :], av[DH : DH + 1, :]
                        )
                        rb0 = rp.tile([1, QT], F32, tag="rb0")
                        nc.sync.dma_start(out=rb0[:], in_=recip[DH : DH + 1, :])
                        rbc = rp.tile([DH, QT], F32, tag="rbc")
                        nc.gpsimd.partition_broadcast(
                            rbc[:], rb0[0:1, :], channels=DH
                        )
                        nc.vector.tensor_mul(
                            headsT[h][:, qs], av[0:DH, :], rbc[:]
                        )

                # ---- output projection (partial over this core's heads) ----
                for tb in range(NTB):
                    ts = slice(tb * P, (tb + 1) * P)
                    stage = outsp.tile([P, D], BF16, tag="stage")
                    for half in range(2):
                        ns = slice(half * 512, (half + 1) * 512)
                        pr = ps_mm.tile([P, 512], F32, tag="mm")
                        for h in range(HPC):
                            nc.tensor.matmul(
                                pr[:],
                                headsT[h][:, ts],
                                wo_sb[h][:, ns],
                                start=(h == 0),
                                stop=(h == HPC - 1),
                            )
                        nc.vector.tensor_copy(stage[:, ns], pr[:])
                    nc.sync.dma_start(
                        out=part[b * S + tb * P : b * S + (tb + 1) * P, :],
                        in_=stage[:],
                    )

            # ---- reduce-scatter partials over token axis; add bias; emit ----
            if collectives:
                nc.gpsimd.collective_compute(
                    "ReduceScatter",
                    mybir.AluOpType.add,
                    replica_groups=GROUPS,
                    ins=[part.opt()],
                    outs=[rs_out.opt()],
                )
            else:
                nc.gpsimd.dma_start(rs_out[:], part[0:TPC, :])
            for tb in range(TPC // P):
                t_in = outsp.tile([P, D], BF16, tag="t_in")
                nc.sync.dma_start(
                    out=t_in[:], in_=rs_out[tb * P : (tb + 1) * P, :]
                )
                tf = outsp.tile([P, D], F32, tag="tf")
                nc.vector.tensor_add(tf[:], t_in[:], bo_sb[:])
                # per-token int8 quantization: q = round(tf * 127/absmax)
                am = rp.tile([P, 1], F32, tag="am")
                nc.vector.tensor_reduce(
                    am[:], tf[:], mybir.AxisListType.X, mybir.AluOpType.max,
                    apply_absolute_value=True,
                )
                nc.vector.tensor_scalar_max(am[:], am[:], 1e-30)
                inv = rp.tile([P, 1], F32, tag="inv")
                nc.vector.reciprocal(inv[:], am[:])
                scl = rp.tile([P, 1], F32, tag="scl")
                nc.vector.tensor_scalar_mul(scl[:], inv[:], 127.0)
                tmp = outsp.tile([P, D], F32, tag="tmp")
                nc.vector.tensor_scalar(
                    tmp[:], tf[:], scl[:], MAGIC,
                    mybir.AluOpType.mult, mybir.AluOpType.add,
                )
                t_q = outsp.tile([P, D], I8, tag="t_q")
                nc.vector.tensor_scalar_add(t_q[:], tmp[:], -MAGIC)
                os_t = rp.tile([P, 1], F32, tag="os_t")
                nc.vector.tensor_scalar_mul(os_t[:], am[:], 1.0 / 127.0)
                nc.sync.dma_start(
                    out=out_d.ap()[tb * P : (tb + 1) * P, 0:D], in_=t_q[:]
                )
                nc.sync.dma_start(
                    out=out_d.ap()[tb * P : (tb + 1) * P, D : D + 4],
                    in_=os_t[:].bitcast(I8),
                )

    nc.compile()
    return nc


# ---------------------------------------------------------------------------
# Host-side runner: cached jit, device-resident weights.
# ---------------------------------------------------------------------------

_STATE = {}


def _make_runner(nc, n_cores):
    install_neuronx_cc_hook()
    partition_name = nc.partition_id_tensor.name if nc.partition_id_tensor else None
    in_names = []
    out_names = []
    out_avals = []
    for alloc in nc.m.functions[0].allocations:
        if not isinstance(alloc, mybir.MemoryLocationSet):
            continue
        name = alloc.memorylocations[0].name
        if alloc.kind == "ExternalInput":
            if name != partition_name:
                in_names.append(name)
        elif alloc.kind == "ExternalOutput":
            out_names.append(name)
            shape = tuple(alloc.tensor_shape)
            dtype = mybir.dt.np(alloc.dtype)
            out_avals.append(jax.core.ShapedArray(shape, dtype))
    n_params = len(in_names)
    all_names = in_names + out_names
    if partition_name is not None:
        all_names.append(partition_name)

    def _body(*args):
        operands = list(args)
        if partition_name is not None:
            operands.append(partition_id_tensor())
        outs = _bass_exec_p.bind(
            *operands,
            out_avals=tuple(out_avals),
            in_names=tuple(all_names),
            out_names=tuple(out_names),
            lowering_input_output_aliases=(),
            sim_require_finite=True,
            sim_require_nnan=True,
            nc=nc,
        )
        return tuple(outs)

    mesh = Mesh(np.asarray(jax.devices()[:n_cores]), ("core",))
    spec = PartitionSpec("core")
    runner = jax.jit(
        shard_map(
            _body,
            mesh=mesh,
            in_specs=(spec,) * (n_params + len(out_names)),
            out_specs=(spec,) * len(out_names),
            check_rep=False,
        ),
        keep_unused=True,
    )
    return runner, in_names, out_avals, mesh


def _get_state():
    if "runner" not in _STATE:
        nc = _build()
        runner, in_names, out_avals, mesh = _make_runner(nc, NCORES)
        _STATE["nc"] = nc
        _STATE["runner"] = runner
        _STATE["in_names"] = in_names
        _STATE["sharding"] = NamedSharding(mesh, PartitionSpec("core"))
        _STATE["devices"] = list(mesh.devices.flat)
        _STATE["zeros"] = [
            jax.device_put(
                np.zeros((NCORES * a.shape[0], *a.shape[1:]), a.dtype),
                _STATE["sharding"],
            )
            for a in out_avals
        ]
    return _STATE


def _stack_weights(w_qkv, b_qkv, w_out):
    wq_l, bq_l, wo0_l, wo1_l = [], [], [], []
    for c in range(NCORES):
        h0 = c * HPC * DH
        cols = [slice(m * D + h0, m * D + h0 + HPC * DH) for m in range(3)]
        wq_l.append(np.concatenate([w_qkv[:, s] for s in cols], axis=1))
        bq_l.append(np.concatenate([b_qkv[s] for s in cols]))
        wo = w_out[h0 : h0 + HPC * DH, :]
        wo0_l.append(wo[:DH])
        wo1_l.append(wo[DH:])
    return (
        np.ascontiguousarray(np.concatenate(wq_l, axis=0), dtype=np.float32),
        np.ascontiguousarray(np.concatenate(bq_l), dtype=np.float32),
        np.ascontiguousarray(np.concatenate(wo0_l, axis=0), dtype=np.float32),
        np.ascontiguousarray(np.concatenate(wo1_l, axis=0), dtype=np.float32),
    )


def _ensure_weights(st, w_qkv, b_qkv, w_out, b_out):
    w_qkv = np.asarray(w_qkv, dtype=np.float32)
    b_qkv = np.asarray(b_qkv, dtype=np.float32)
    w_out = np.asarray(w_out, dtype=np.float32)
    b_out = np.asarray(b_out, dtype=np.float32)
    cached = st.get("w_host")
    if cached is not None and all(
        np.array_equal(a, b)
        for a, b in zip(cached, (w_qkv, b_qkv, w_out, b_out))
    ):
        return
    wq, bq, wo0, wo1 = _stack_weights(w_qkv, b_qkv, w_out)
    bo = np.tile(b_out[None, :], (NCORES, 1))
    sh = st["sharding"]
    st["w_dev"] = {
        "w_qkv_shard": jax.device_put(wq, sh),
        "b_qkv_shard": jax.device_put(bq, sh),
        "w_out0": jax.device_put(wo0, sh),
        "w_out1": jax.device_put(wo1, sh),
        "b_out": jax.device_put(bo, sh),
    }
    st["w_host"] = (w_qkv, b_qkv, w_out, b_out)


def _pack_slice(xf):
    """Quantize one [n, D] fp32 slice to packed int8 [n, XW]."""
    n = xf.shape[0]
    xg = xf.reshape(n, NG, GS)
    am = np.maximum(xg.max(axis=2), -xg.min(axis=2))
    np.maximum(am, np.float32(1e-30), out=am)
    inv = np.float32(127.0) / am
    q = np.rint(xg * inv[:, :, None]).astype(np.int8).reshape(n, D)
    buf = np.empty((n, XW), dtype=np.int8)
    buf[:, :D] = q
    buf[:, D:] = (am * np.float32(1.0 / 127.0)).view(np.int8)
    return buf


def _pack_x(x):
    xf = np.ascontiguousarray(np.asarray(x, dtype=np.float32)).reshape(TOK, D)
    if not X_INT8:
        return xf.astype(np.float16)
    return _pack_slice(xf)


def _upload_x(x, st):
    """Cast per-core slices and start each device transfer immediately, so
    the fp16 cast of slice c+1 overlaps the upload of slice c."""
    from jax.sharding import SingleDeviceSharding

    xf = np.asarray(x, dtype=np.float32).reshape(TOK, D)
    devs = st["devices"]
    parts = []
    for c in range(NCORES):
        sl = xf[c * TPC : (c + 1) * TPC]
        sl = _pack_slice(sl) if X_INT8 else sl.astype(np.float16)
        parts.append(jax.device_put(sl, SingleDeviceSharding(devs[c])))
    return jax.make_array_from_single_device_arrays(
        (TOK, XW if X_INT8 else D), st["sharding"], parts
    )


def kernel(x, w_qkv, b_qkv, w_out, b_out):
    st = _get_state()
    _ensure_weights(st, w_qkv, b_qkv, w_out, b_out)
    xd = _upload_x(x, st)
    args = []
    for name in st["in_names"]:
        if name == "x_shard":
            args.append(xd)
        else:
            args.append(st["w_dev"][name])
    outs = st["runner"](*args, *st["zeros"])
    out = np.empty((TOK, D), dtype=np.float32)
    shards = sorted(
        outs[0].addressable_shards, key=lambda s: s.index[0].start or 0
    )
    for s in shards:
        s.data.copy_to_host_async()
    for s in shards:
        raw = np.asarray(s.data)
        r0 = s.index[0].start or 0
        q = raw[:, :D]
        osc = np.ascontiguousarray(raw[:, D:]).view(np.float32)
        np.multiply(q, osc, out=out[r0 : r0 + raw.shape[0]])
    return out.reshape(B, S, D)
